# revision 25
# baseline (speedup 1.0000x reference)
"""Causal self-attention (B=2, T=2048, C=1024, 16 heads) on 8 trn2 cores.

Sharding: core = (batch b, head-group hg) on a 2x4 grid; each core computes
QKV projection, causal attention and the partial c_proj for its 4 heads of
one batch element. Host sums the 4 partials per batch element (replaces the
all-reduce) and adds bproj.

Device layout per core (all matmuls float32r, free dim >= 256):
  - x arrives host-transposed as xT [C=1024, T=2048] so C is the
    contraction dim on partitions for both QKV forms.
  - Q^T/K^T computed as [qkv_col, t] tiles (head h lives at partitions
    (h%2)*64..) via matmul(lhsT=W_slice, rhs=xT).
  - V computed as [t, v_col] tiles via matmul(lhsT=xT, rhs=Wv), stored
    with a fused ones column per head ([V_h | 1] -> M=65) so the AV
    matmul's row 64 accumulates the softmax denominator.
  - Scores computed transposed: S^T[k, q] = matmul(lhsT=K^T_ktile,
    rhs=Q^T_qblock); causal triangle masked additively on PSUM; exp on
    ScalarE (scale=1/8) into SBUF; AV accumulates O^T[d, q] over k-tiles.
  - O^T scaled by the reciprocal of the denominator row (gpsimd
    partition-broadcast) + V-bias folded in (exact: softmax rows sum to 1),
    written into y^T [head_channel, t]; partial out = matmul(lhsT=y^T,
    rhs=Wproj_rows) per t-tile.
"""

import os
import sys
import types

import numpy as np

# ---------------------------------------------------------------------------
# Environment compatibility (self-contained on purpose).
# ---------------------------------------------------------------------------


def _install_axon_ntff_hook():
    """Provide the missing ``antenv.axon_hooks`` module so that
    ``run_bass_kernel_spmd(trace=True)`` works under axon in this container."""
    if "antenv.axon_hooks" in sys.modules:
        return
    try:
        import antenv
    except ImportError:
        return
    mod = types.ModuleType("antenv.axon_hooks")
    holder = [None]
    mod.set_axon_ntff_profile_hook = lambda h: holder.__setitem__(0, h)
    mod.get_axon_ntff_profile_hook = lambda: holder[0]
    sys.modules["antenv.axon_hooks"] = mod
    antenv.axon_hooks = mod
    try:
        from trn_agent_boot.trn_boot import _ntff_profile_via_ctypes

        hook = _ntff_profile_via_ctypes("/opt/axon/libaxon_pjrt.so")
        if hook is not None:
            mod.set_axon_ntff_profile_hook(hook)
    except Exception:
        pass


_install_axon_ntff_hook()

import concourse.bass as bass  # noqa: E402
import concourse.mybir as mybir  # noqa: E402
import concourse.tile as tile  # noqa: E402
from concourse.bass_utils import run_bass_kernel_spmd  # noqa: E402


def _split_multi_waits(nc, max_waits=1):
    """The walrus build here rejects instructions with more than one sync
    wait; move excess waits onto same-engine NoOps placed just before the
    instruction (sequential waiting is equivalent for monotonic sems)."""
    n = 0
    for func in nc.m.functions:
        for bb in func.blocks:
            out = []
            changed = False
            for inst in bb.instructions:
                si = inst.sync_info
                waits = list(si.on_wait) if si is not None and si.on_wait else []
                if len(waits) > max_waits:
                    changed = True
                    extra, keep = waits[:-max_waits], waits[-max_waits:]
                    for i in range(0, len(extra), max_waits):
                        n += 1
                        out.append(
                            mybir.InstNoOp(
                                name=f"{inst.name}-ws{i}",
                                engine=inst.engine,
                                ins=[],
                                outs=[],
                                sync_info=mybir.SyncInfo(
                                    on_wait=extra[i : i + max_waits], on_update=[]
                                ),
                                text_hint="wait_split",
                            )
                        )
                    si.on_wait = keep
                out.append(inst)
            if changed:
                bb.instructions = out
    return n


# ---------------------------------------------------------------------------
# Problem constants (hardcoded per spec).
# ---------------------------------------------------------------------------

B, T, C = 2, 2048, 1024
N_HEAD = 16
D = 64  # head dim
N_CORES = 8
HG = 4  # head groups (cores per batch element)
NH = N_HEAD // HG  # heads per core = 4
HD = NH * D  # head channels per core = 256
CK = C // 128  # contraction chunks = 8
TT = T // 128  # t tiles = 16
QB = T // 512  # q blocks = 4

F32 = mybir.dt.float32
_MM_CHOICES = {
    "fp32": mybir.dt.float32,
    "fp32r": mybir.dt.float32r,
    "bf16": mybir.dt.bfloat16,
}
MM_DT = _MM_CHOICES[os.environ.get("KERNEL_MM_DT", "bf16")]
MM_NP = mybir.dt.np(MM_DT)

NEG = -1.0e9

TRACE = False
LAST_RESULT = None
_NC_CACHE = {}


def _mm(ap):
    return ap


def _build_nc():
    nc = bass.Bass("TRN2", target_bir_lowering=False)

    xT = nc.dram_tensor("xT", [C, T], MM_DT, kind="ExternalInput")
    wq = nc.dram_tensor("wq", [C, HD], MM_DT, kind="ExternalInput")
    wk = nc.dram_tensor("wk", [C, HD], MM_DT, kind="ExternalInput")
    wv = nc.dram_tensor("wv", [C, HD], MM_DT, kind="ExternalInput")
    bq = nc.dram_tensor("bq", [128, NH], F32, kind="ExternalInput")
    bk = nc.dram_tensor("bk", [128, HD // 128], F32, kind="ExternalInput")
    bv = nc.dram_tensor("bv", [HD], F32, kind="ExternalInput")
    wp = nc.dram_tensor("wp", [HD, C], MM_DT, kind="ExternalInput")
    out = nc.dram_tensor("out", [T, C], F32, kind="ExternalOutput")
    ones_dram = nc.inline_tensor(np.ones((128, NH, 1), MM_NP), name="ones_col")

    with tile.TileContext(nc) as tc:
        _emit(nc, tc, xT, wq, wk, wv, bq, bk, bv, wp, out, ones_dram)

    _split_multi_waits(nc)
    return nc


def _emit(nc, tc, xT, wq, wk, wv, bq, bk, bv, wp, out, ones_dram):
    from contextlib import ExitStack

    ctx = ExitStack()
    with ctx:
        consts = ctx.enter_context(tc.tile_pool(name="consts", bufs=1))
        xt_pool = ctx.enter_context(tc.tile_pool(name="xt", bufs=CK))
        qz_pool = ctx.enter_context(tc.tile_pool(name="qz", bufs=NH))
        kt_pool = ctx.enter_context(tc.tile_pool(name="kt", bufs=HD // 128))
        vo_pool = ctx.enter_context(tc.tile_pool(name="vo", bufs=TT))
        yt_pool = ctx.enter_context(tc.tile_pool(name="yt", bufs=2))
        pt_pool = ctx.enter_context(tc.tile_pool(name="pt", bufs=8))
        rb_pool = ctx.enter_context(tc.tile_pool(name="rb", bufs=2))
        dram = ctx.enter_context(tc.tile_pool(name="dram", bufs=2, space="DRAM"))
        ob_pool = ctx.enter_context(tc.tile_pool(name="ob", bufs=3))
        # p_qk is shared between the QKV phase and the c_proj phase (same tag)
        p_qk = ctx.enter_context(tc.tile_pool(name="p_qk", bufs=2, space="PSUM"))
        p_st = ctx.enter_context(tc.tile_pool(name="p_st", bufs=3, space="PSUM"))
        p_ot = ctx.enter_context(tc.tile_pool(name="p_ot", bufs=3, space="PSUM"))
        p_pr = p_qk

        # ---- constant loads (ordered so the first QK matmul unblocks asap) --
        bq_sb = consts.tile([128, NH], F32, tag="bq")
        nc.sync.dma_start(bq_sb[:], bq[:])
        bk_sb = consts.tile([128, HD // 128], F32, tag="bk")
        nc.sync.dma_start(bk_sb[:], bk[:])
        bv_sb = consts.tile([128, HD // 128], F32, tag="bv")
        nc.sync.dma_start(bv_sb[:], bv.rearrange("(o p) -> p o", p=128))

        w_sb = {}

        def load_w(name, w):
            t = consts.tile([128, CK, HD], MM_DT, tag=name, name=name)
            nc.sync.dma_start(t[:], w.rearrange("(o p) n -> p o n", p=128))
            w_sb[name] = t

        wq_t = consts.tile([128, CK, HD], MM_DT, tag="wq", name="wq")
        w_sb["wq"] = wq_t
        wq_r = wq.rearrange("(o p) n -> p o n", p=128)
        nc.sync.dma_start(wq_t[:, 0:2], wq_r[:, 0:2])
        xt_sb = []
        for ck in range(CK):
            t = xt_pool.tile([128, T], MM_DT, tag="xt", name=f"xt{ck}")
            nc.sync.dma_start(t[:], xT[ck * 128 : (ck + 1) * 128, :])
            xt_sb.append(t)
            if ck == 0:
                nc.sync.dma_start(wq_t[:, 2:CK], wq_r[:, 2:CK])
                load_w("wk", wk)
        load_w("wv", wv)
        wp_sb = consts.tile([128, HD // 128, C], MM_DT, tag="wp")
        nc.sync.dma_start(wp_sb[:], wp.rearrange("(o p) n -> p o n", p=128))

        # additive causal triangle mask [k_rel, q_rel]: 0 where k<=q else NEG
        mask_sb = consts.tile([128, 128], F32, tag="mask")
        nc.gpsimd.memset(mask_sb[:], 0.0)
        nc.gpsimd.affine_select(
            out=mask_sb[:],
            in_=mask_sb[:],
            compare_op=mybir.AluOpType.is_ge,
            fill=NEG,
            base=0,
            pattern=[[1, 128]],
            channel_multiplier=-1,
        )

        # ---- QKV projection -------------------------------------------------
        # K^T pair tiles [128, T]: tile i = heads (2i, 2i+1) stacked 64+64.
        # Q^T zero-padded per-head tiles [128, T]: rows 0-63 = Q_h, rows
        # 64-127 = 0, so score matmuls contract over the full 128 partitions
        # (lhsT = K pair tile, the other head's K rows hit zeros).
        qz_sb = [
            qz_pool.tile([128, T], MM_DT, tag="qz", name=f"qz{h}") for h in range(NH)
        ]
        kt_sb = [
            kt_pool.tile([128, T], MM_DT, tag="kt", name=f"kt{i}")
            for i in range(HD // 128)
        ]
        for h in range(NH):
            zb = 64 - (h % 2) * 64  # zero the half NOT holding Q_h
            nc.gpsimd.memset(qz_sb[h][zb : zb + 64, :], 0.0)

        for i in range(HD // 128):
            for tb in range(QB):
                tbc = slice(tb * 512, (tb + 1) * 512)
                ps = p_qk.tile([128, 512], F32, tag="pq")
                for ck in range(CK):
                    nc.tensor.matmul(
                        ps[:],
                        _mm(w_sb["wq"][:, ck, i * 128 : (i + 1) * 128]),
                        _mm(xt_sb[ck][:, tbc]),
                        start=(ck == 0),
                        stop=(ck == CK - 1),
                    )
                for hh in (2 * i, 2 * i + 1):
                    hb = (hh % 2) * 64
                    nc.vector.tensor_scalar(
                        qz_sb[hh][hb : hb + 64, tbc],
                        ps[hb : hb + 64, :],
                        bq_sb[hb : hb + 64, hh : hh + 1],
                        None,
                        mybir.AluOpType.add,
                    )
        for i in range(HD // 128):
            for tb in range(QB):
                tbc = slice(tb * 512, (tb + 1) * 512)
                ps = p_qk.tile([128, 512], F32, tag="pq")
                for ck in range(CK):
                    nc.tensor.matmul(
                        ps[:],
                        _mm(w_sb["wk"][:, ck, i * 128 : (i + 1) * 128]),
                        _mm(xt_sb[ck][:, tbc]),
                        start=(ck == 0),
                        stop=(ck == CK - 1),
                    )
                nc.vector.tensor_scalar(
                    kt_sb[i][:, tbc],
                    ps[:],
                    bk_sb[:, i : i + 1],
                    None,
                    mybir.AluOpType.add,
                )

        # V tiles [128, NH*128]: per head [V(64) | ones | zeros(63)] so the
        # PV matmul runs with a full 128-wide stationary operand; psum rows
        # 65-127 accumulate zeros and are ignored. Emitted per q-block inside
        # the attention loop so exp work starts overlapping the projections.
        vo_sb = [None] * TT

        def emit_v_tile(tt):
            t = vo_pool.tile([128, NH * 128], MM_DT, tag="vo", name=f"vo{tt}")
            vo_sb[tt] = t
            v4 = t[:].rearrange("p (h c) -> p h c", h=NH)
            nc.gpsimd.memset(v4[:, :, D + 1 :], 0.0)
            nc.sync.dma_start(v4[:, :, D : D + 1], ones_dram[:])
            ps = p_qk.tile([128, 512], F32, tag="pq")
            for ck in range(CK):
                nc.tensor.matmul(
                    ps[:, :HD],
                    _mm(xt_sb[ck][:, tt * 128 : (tt + 1) * 128]),
                    _mm(w_sb["wv"][:, ck, :]),
                    start=(ck == 0),
                    stop=(ck == CK - 1),
                )
            nc.vector.tensor_copy(
                v4[:, :, 0:D],
                ps[:, :HD].rearrange("p (h c) -> p h c", h=NH),
            )

        # ---- attention ------------------------------------------------------
        yt_sb = [
            yt_pool.tile([128, T], MM_DT, tag="yt", name=f"yt{g}")
            for g in range(HD // 128)
        ]

        for tt in range(TT):
            emit_v_tile(tt)

        for qb in range(QB):
            q0 = qb * 512
            for h in range(NH):
                i, jb = h // 2, (h % 2) * 64
                kd = kt_sb[i]
                qd = qz_sb[h]
                ot = p_ot.tile([128, 512], F32, tag="ot")
                n_kt = 4 * qb + 4

                def emit_st_exp(kt):
                    j = kt - 4 * qb
                    if j < 0:
                        c_mm, c_exp = 0, 0
                    elif j < 3:
                        c_mm = c_exp = 128 * j
                    else:
                        c_mm, c_exp = 256, 384
                    st = p_st.tile([128, 512], F32, tag="st")
                    nc.tensor.matmul(
                        st[:, c_mm:512],
                        _mm(kd[:, kt * 128 : (kt + 1) * 128]),
                        _mm(qd[:, q0 + c_mm : q0 + 512]),
                        start=True,
                        stop=True,
                    )
                    pt = pt_pool.tile([128, 512], MM_DT, tag="pt")
                    if j >= 0:
                        # triangle mask for the diagonal 128-col window
                        nc.vector.tensor_tensor(
                            st[:, c_exp : c_exp + 128],
                            st[:, c_exp : c_exp + 128],
                            mask_sb[:],
                            mybir.AluOpType.add,
                        )
                    nc.scalar.activation(
                        pt[:, c_exp:512],
                        st[:, c_exp:512],
                        mybir.ActivationFunctionType.Exp,
                        scale=0.125,
                    )
                    return pt, c_exp

                def emit_av(kt, pt, c_av):
                    nc.tensor.matmul(
                        ot[:, c_av:512],
                        _mm(vo_sb[kt][:, h * 128 : (h + 1) * 128]),
                        _mm(pt[:, c_av:512]),
                        start=(kt == 0),
                        stop=(kt == n_kt - 1),
                    )

                pending = []
                for kt in range(n_kt):
                    pt, c_av = emit_st_exp(kt)
                    pending.append((kt, pt, c_av))
                    if len(pending) > 2:
                        emit_av(*pending.pop(0))
                for p in pending:
                    emit_av(*p)

                # normalize + V bias, write y^T slice.
                # The denominator row is bounced through DRAM to fold it to
                # [128, 4] so the (slow, ~8 cyc/elem) exact reciprocal runs on
                # all 128 lanes, then unfolded + partition-broadcast back.
                sums = rb_pool.tile([1, 512], F32, tag="sums")
                nc.vector.tensor_copy(sums[:], ot[64:65, :])
                rc_d = dram.tile([1, 512], F32, tag="rc_d")
                nc.sync.dma_start(rc_d[:], sums[:])
                r4 = rb_pool.tile([128, 4], F32, tag="r4")
                nc.sync.dma_start(r4[:], rc_d[0, :].rearrange("(p o) -> p o", p=128))
                nc.vector.reciprocal(r4[:], r4[:])
                rc2_d = dram.tile([1, 512], F32, tag="rc2_d")
                nc.sync.dma_start(
                    rc2_d[0, :].rearrange("(p o) -> p o", p=128), r4[:]
                )
                rb = rb_pool.tile([64, 512], F32, tag="rb")
                nc.sync.dma_start(rb[:], rc2_d[:].to_broadcast((64, 512)))
                yslice = yt_sb[i][jb : jb + 64, q0 : q0 + 512]
                nc.vector.tensor_tensor(
                    yslice,
                    ot[0:64, :],
                    rb[:],
                    mybir.AluOpType.mult,
                )
                nc.vector.tensor_scalar(
                    yslice,
                    yslice,
                    bv_sb[jb : jb + 64, h // 2 : h // 2 + 1],
                    None,
                    mybir.AluOpType.add,
                )

            # partial c_proj for this q-block's t-tiles: dense full-array
            # matmuls interleaved into the exp-bound attention stretch.
            for tt in range(qb * 4, qb * 4 + 4):
                for nb in range(C // 512):
                    ps = p_pr.tile([128, 512], F32, tag="pq")
                    for g in range(HD // 128):
                        nc.tensor.matmul(
                            ps[:],
                            _mm(yt_sb[g][:, tt * 128 : (tt + 1) * 128]),
                            _mm(wp_sb[:, g, nb * 512 : (nb + 1) * 512]),
                            start=(g == 0),
                            stop=(g == HD // 128 - 1),
                        )
                    ob = ob_pool.tile([128, 512], F32, tag="ob")
                    nc.vector.tensor_copy(ob[:], ps[:])
                    nc.sync.dma_start(
                        out[tt * 128 : (tt + 1) * 128, nb * 512 : (nb + 1) * 512],
                        ob[:],
                    )


def _get_nc():
    key = str(MM_DT)
    if key not in _NC_CACHE:
        _NC_CACHE[key] = _build_nc()
    return _NC_CACHE[key]


def _dup_bias(b):
    # [NH*64] -> [128, NH]: head h's 64 biases replicated on both halves
    m = b.reshape(NH, 64).T  # [64, NH]
    return np.ascontiguousarray(np.vstack([m, m]).astype(np.float32))


def kernel(x, Wqkv, bqkv, Wproj, bproj):
    global LAST_RESULT
    x = np.asarray(x, dtype=np.float32)
    Wqkv = np.asarray(Wqkv, dtype=np.float32)
    bqkv = np.asarray(bqkv, dtype=np.float32)
    Wproj = np.asarray(Wproj, dtype=np.float32)
    bproj = np.asarray(bproj, dtype=np.float32)

    nc = _get_nc()
    in_maps = []
    for core in range(N_CORES):
        b, hg = core // HG, core % HG
        cs, ce = hg * HD, (hg + 1) * HD
        in_maps.append(
            {
                "xT": np.ascontiguousarray(x[b].T.astype(MM_NP)),
                "wq": np.ascontiguousarray(Wqkv[:, cs:ce].astype(MM_NP)),
                "wk": np.ascontiguousarray(Wqkv[:, C + cs : C + ce].astype(MM_NP)),
                "wv": np.ascontiguousarray(
                    Wqkv[:, 2 * C + cs : 2 * C + ce].astype(MM_NP)
                ),
                "bq": _dup_bias(bqkv[cs:ce]),
                "bk": np.ascontiguousarray(
                    bqkv[C + cs : C + ce].reshape(2, 128).T.astype(np.float32)
                ),
                "bv": np.ascontiguousarray(bqkv[2 * C + cs : 2 * C + ce]),
                "wp": np.ascontiguousarray(Wproj[cs:ce, :].astype(MM_NP)),
            }
        )

    res = run_bass_kernel_spmd(
        nc, in_maps, core_ids=list(range(N_CORES)), trace=TRACE
    )
    LAST_RESULT = res

    outp = np.empty((B, T, C), dtype=np.float32)
    for b in range(B):
        acc = res.results[b * HG]["out"].astype(np.float32).copy()
        for hg in range(1, HG):
            acc += res.results[b * HG + hg]["out"]
        outp[b] = acc + bproj
    return outp


# revision 26
# speedup vs baseline: 1.0113x; 1.0113x over previous
"""Causal self-attention (B=2, T=2048, C=1024, 16 heads) on 8 trn2 cores.

Sharding: core = (batch b, head-group hg) on a 2x4 grid; each core computes
QKV projection, causal attention and the partial c_proj for its 4 heads of
one batch element. Host sums the 4 partials per batch element (replaces the
all-reduce) and adds bproj.

Device layout per core (all matmuls float32r, free dim >= 256):
  - x arrives host-transposed as xT [C=1024, T=2048] so C is the
    contraction dim on partitions for both QKV forms.
  - Q^T/K^T computed as [qkv_col, t] tiles (head h lives at partitions
    (h%2)*64..) via matmul(lhsT=W_slice, rhs=xT).
  - V computed as [t, v_col] tiles via matmul(lhsT=xT, rhs=Wv), stored
    with a fused ones column per head ([V_h | 1] -> M=65) so the AV
    matmul's row 64 accumulates the softmax denominator.
  - Scores computed transposed: S^T[k, q] = matmul(lhsT=K^T_ktile,
    rhs=Q^T_qblock); causal triangle masked additively on PSUM; exp on
    ScalarE (scale=1/8) into SBUF; AV accumulates O^T[d, q] over k-tiles.
  - O^T scaled by the reciprocal of the denominator row (gpsimd
    partition-broadcast) + V-bias folded in (exact: softmax rows sum to 1),
    written into y^T [head_channel, t]; partial out = matmul(lhsT=y^T,
    rhs=Wproj_rows) per t-tile.
"""

import os
import sys
import types

import numpy as np

# ---------------------------------------------------------------------------
# Environment compatibility (self-contained on purpose).
# ---------------------------------------------------------------------------


def _install_axon_ntff_hook():
    """Provide the missing ``antenv.axon_hooks`` module so that
    ``run_bass_kernel_spmd(trace=True)`` works under axon in this container."""
    if "antenv.axon_hooks" in sys.modules:
        return
    try:
        import antenv
    except ImportError:
        return
    mod = types.ModuleType("antenv.axon_hooks")
    holder = [None]
    mod.set_axon_ntff_profile_hook = lambda h: holder.__setitem__(0, h)
    mod.get_axon_ntff_profile_hook = lambda: holder[0]
    sys.modules["antenv.axon_hooks"] = mod
    antenv.axon_hooks = mod
    try:
        from trn_agent_boot.trn_boot import _ntff_profile_via_ctypes

        hook = _ntff_profile_via_ctypes("/opt/axon/libaxon_pjrt.so")
        if hook is not None:
            mod.set_axon_ntff_profile_hook(hook)
    except Exception:
        pass


_install_axon_ntff_hook()

import concourse.bass as bass  # noqa: E402
import concourse.mybir as mybir  # noqa: E402
import concourse.tile as tile  # noqa: E402
from concourse.bass_utils import run_bass_kernel_spmd  # noqa: E402


def _split_multi_waits(nc, max_waits=1):
    """The walrus build here rejects instructions with more than one sync
    wait; move excess waits onto same-engine NoOps placed just before the
    instruction (sequential waiting is equivalent for monotonic sems)."""
    n = 0
    for func in nc.m.functions:
        for bb in func.blocks:
            out = []
            changed = False
            for inst in bb.instructions:
                si = inst.sync_info
                waits = list(si.on_wait) if si is not None and si.on_wait else []
                if len(waits) > max_waits:
                    changed = True
                    extra, keep = waits[:-max_waits], waits[-max_waits:]
                    for i in range(0, len(extra), max_waits):
                        n += 1
                        out.append(
                            mybir.InstNoOp(
                                name=f"{inst.name}-ws{i}",
                                engine=inst.engine,
                                ins=[],
                                outs=[],
                                sync_info=mybir.SyncInfo(
                                    on_wait=extra[i : i + max_waits], on_update=[]
                                ),
                                text_hint="wait_split",
                            )
                        )
                    si.on_wait = keep
                out.append(inst)
            if changed:
                bb.instructions = out
    return n


# ---------------------------------------------------------------------------
# Problem constants (hardcoded per spec).
# ---------------------------------------------------------------------------

B, T, C = 2, 2048, 1024
N_HEAD = 16
D = 64  # head dim
N_CORES = 8
HG = 4  # head groups (cores per batch element)
NH = N_HEAD // HG  # heads per core = 4
HD = NH * D  # head channels per core = 256
CK = C // 128  # contraction chunks = 8
TT = T // 128  # t tiles = 16
QB = T // 512  # q blocks = 4

F32 = mybir.dt.float32
_MM_CHOICES = {
    "fp32": mybir.dt.float32,
    "fp32r": mybir.dt.float32r,
    "bf16": mybir.dt.bfloat16,
}
MM_DT = _MM_CHOICES[os.environ.get("KERNEL_MM_DT", "bf16")]
MM_NP = mybir.dt.np(MM_DT)

NEG = -1.0e9

TRACE = False
LAST_RESULT = None
_NC_CACHE = {}


def _mm(ap):
    return ap


def _build_nc():
    nc = bass.Bass("TRN2", target_bir_lowering=False)

    xT = nc.dram_tensor("xT", [C, T], MM_DT, kind="ExternalInput")
    wq = nc.dram_tensor("wq", [C, HD], MM_DT, kind="ExternalInput")
    wk = nc.dram_tensor("wk", [C, HD], MM_DT, kind="ExternalInput")
    wv = nc.dram_tensor("wv", [C, HD], MM_DT, kind="ExternalInput")
    bq = nc.dram_tensor("bq", [128, NH], F32, kind="ExternalInput")
    bk = nc.dram_tensor("bk", [128, HD // 128], F32, kind="ExternalInput")
    bv = nc.dram_tensor("bv", [HD], F32, kind="ExternalInput")
    wp = nc.dram_tensor("wp", [HD, C], MM_DT, kind="ExternalInput")
    out = nc.dram_tensor("out", [T, C], F32, kind="ExternalOutput")
    ones_dram = nc.inline_tensor(np.ones((128, NH, 1), MM_NP), name="ones_col")

    with tile.TileContext(nc) as tc:
        _emit(nc, tc, xT, wq, wk, wv, bq, bk, bv, wp, out, ones_dram)

    _split_multi_waits(nc)
    return nc


def _emit(nc, tc, xT, wq, wk, wv, bq, bk, bv, wp, out, ones_dram):
    from contextlib import ExitStack

    ctx = ExitStack()
    with ctx:
        consts = ctx.enter_context(tc.tile_pool(name="consts", bufs=1))
        xt_pool = ctx.enter_context(tc.tile_pool(name="xt", bufs=CK))
        qz_pool = ctx.enter_context(tc.tile_pool(name="qz", bufs=NH))
        kt_pool = ctx.enter_context(tc.tile_pool(name="kt", bufs=HD // 128))
        vo_pool = ctx.enter_context(tc.tile_pool(name="vo", bufs=TT))
        yt_pool = ctx.enter_context(tc.tile_pool(name="yt", bufs=2))
        pt_pool = ctx.enter_context(tc.tile_pool(name="pt", bufs=10))
        rb_pool = ctx.enter_context(tc.tile_pool(name="rb", bufs=2))
        dram = ctx.enter_context(tc.tile_pool(name="dram", bufs=2, space="DRAM"))
        ob_pool = ctx.enter_context(tc.tile_pool(name="ob", bufs=3))
        # p_qk is shared between the QKV phase and the c_proj phase (same tag)
        p_qk = ctx.enter_context(tc.tile_pool(name="p_qk", bufs=2, space="PSUM"))
        p_st = ctx.enter_context(tc.tile_pool(name="p_st", bufs=3, space="PSUM"))
        p_ot = ctx.enter_context(tc.tile_pool(name="p_ot", bufs=3, space="PSUM"))
        p_pr = p_qk

        # ---- constant loads (ordered so the first QK matmul unblocks asap) --
        bq_sb = consts.tile([128, NH], F32, tag="bq")
        nc.sync.dma_start(bq_sb[:], bq[:])
        bk_sb = consts.tile([128, HD // 128], F32, tag="bk")
        nc.sync.dma_start(bk_sb[:], bk[:])
        bv_sb = consts.tile([128, HD // 128], F32, tag="bv")
        nc.sync.dma_start(bv_sb[:], bv.rearrange("(o p) -> p o", p=128))

        w_sb = {}

        def load_w(name, w):
            t = consts.tile([128, CK, HD], MM_DT, tag=name, name=name)
            nc.sync.dma_start(t[:], w.rearrange("(o p) n -> p o n", p=128))
            w_sb[name] = t

        wq_t = consts.tile([128, CK, HD], MM_DT, tag="wq", name="wq")
        w_sb["wq"] = wq_t
        wq_r = wq.rearrange("(o p) n -> p o n", p=128)
        nc.sync.dma_start(wq_t[:, 0:2], wq_r[:, 0:2])
        xt_sb = []
        for ck in range(CK):
            t = xt_pool.tile([128, T], MM_DT, tag="xt", name=f"xt{ck}")
            nc.sync.dma_start(t[:], xT[ck * 128 : (ck + 1) * 128, :])
            xt_sb.append(t)
            if ck == 0:
                nc.sync.dma_start(wq_t[:, 2:CK], wq_r[:, 2:CK])
                load_w("wk", wk)
        load_w("wv", wv)
        wp_sb = consts.tile([128, HD // 128, C], MM_DT, tag="wp")
        nc.sync.dma_start(wp_sb[:], wp.rearrange("(o p) n -> p o n", p=128))

        # additive causal triangle mask [k_rel, q_rel]: 0 where k<=q else NEG
        mask_sb = consts.tile([128, 128], F32, tag="mask")
        nc.gpsimd.memset(mask_sb[:], 0.0)
        nc.gpsimd.affine_select(
            out=mask_sb[:],
            in_=mask_sb[:],
            compare_op=mybir.AluOpType.is_ge,
            fill=NEG,
            base=0,
            pattern=[[1, 128]],
            channel_multiplier=-1,
        )

        # ---- QKV projection -------------------------------------------------
        # K^T pair tiles [128, T]: tile i = heads (2i, 2i+1) stacked 64+64.
        # Q^T zero-padded per-head tiles [128, T]: rows 0-63 = Q_h, rows
        # 64-127 = 0, so score matmuls contract over the full 128 partitions
        # (lhsT = K pair tile, the other head's K rows hit zeros).
        qz_sb = [
            qz_pool.tile([128, T], MM_DT, tag="qz", name=f"qz{h}") for h in range(NH)
        ]
        kt_sb = [
            kt_pool.tile([128, T], MM_DT, tag="kt", name=f"kt{i}")
            for i in range(HD // 128)
        ]
        for h in range(NH):
            zb = 64 - (h % 2) * 64  # zero the half NOT holding Q_h
            nc.gpsimd.memset(qz_sb[h][zb : zb + 64, :], 0.0)

        for i in range(HD // 128):
            for tb in range(QB):
                tbc = slice(tb * 512, (tb + 1) * 512)
                ps = p_qk.tile([128, 512], F32, tag="pq")
                for ck in range(CK):
                    nc.tensor.matmul(
                        ps[:],
                        _mm(w_sb["wq"][:, ck, i * 128 : (i + 1) * 128]),
                        _mm(xt_sb[ck][:, tbc]),
                        start=(ck == 0),
                        stop=(ck == CK - 1),
                    )
                for hh in (2 * i, 2 * i + 1):
                    hb = (hh % 2) * 64
                    nc.vector.tensor_scalar(
                        qz_sb[hh][hb : hb + 64, tbc],
                        ps[hb : hb + 64, :],
                        bq_sb[hb : hb + 64, hh : hh + 1],
                        None,
                        mybir.AluOpType.add,
                    )
        for i in range(HD // 128):
            for tb in range(QB):
                tbc = slice(tb * 512, (tb + 1) * 512)
                ps = p_qk.tile([128, 512], F32, tag="pq")
                for ck in range(CK):
                    nc.tensor.matmul(
                        ps[:],
                        _mm(w_sb["wk"][:, ck, i * 128 : (i + 1) * 128]),
                        _mm(xt_sb[ck][:, tbc]),
                        start=(ck == 0),
                        stop=(ck == CK - 1),
                    )
                nc.vector.tensor_scalar(
                    kt_sb[i][:, tbc],
                    ps[:],
                    bk_sb[:, i : i + 1],
                    None,
                    mybir.AluOpType.add,
                )

        # V tiles [128, NH*128]: per head [V(64) | ones | zeros(63)] so the
        # PV matmul runs with a full 128-wide stationary operand; psum rows
        # 65-127 accumulate zeros and are ignored. Emitted per q-block inside
        # the attention loop so exp work starts overlapping the projections.
        vo_sb = [None] * TT

        def emit_v_tile(tt):
            t = vo_pool.tile([128, NH * 128], MM_DT, tag="vo", name=f"vo{tt}")
            vo_sb[tt] = t
            v4 = t[:].rearrange("p (h c) -> p h c", h=NH)
            nc.gpsimd.memset(v4[:, :, D + 1 :], 0.0)
            nc.sync.dma_start(v4[:, :, D : D + 1], ones_dram[:])
            ps = p_qk.tile([128, 512], F32, tag="pq")
            for ck in range(CK):
                nc.tensor.matmul(
                    ps[:, :HD],
                    _mm(xt_sb[ck][:, tt * 128 : (tt + 1) * 128]),
                    _mm(w_sb["wv"][:, ck, :]),
                    start=(ck == 0),
                    stop=(ck == CK - 1),
                )
            nc.vector.tensor_copy(
                v4[:, :, 0:D],
                ps[:, :HD].rearrange("p (h c) -> p h c", h=NH),
            )

        # ---- attention ------------------------------------------------------
        yt_sb = [
            yt_pool.tile([128, T], MM_DT, tag="yt", name=f"yt{g}")
            for g in range(HD // 128)
        ]

        for tt in range(TT):
            emit_v_tile(tt)

        for qb in range(QB):
            q0 = qb * 512
            for h in range(NH):
                i, jb = h // 2, (h % 2) * 64
                kd = kt_sb[i]
                qd = qz_sb[h]
                ot = p_ot.tile([128, 512], F32, tag="ot")
                n_kt = 4 * qb + 4

                def emit_st_exp(kt):
                    j = kt - 4 * qb
                    if j < 0:
                        c_mm, c_exp = 0, 0
                    elif j < 3:
                        c_mm = c_exp = 128 * j
                    else:
                        c_mm, c_exp = 256, 384
                    st = p_st.tile([128, 512], F32, tag="st")
                    nc.tensor.matmul(
                        st[:, c_mm:512],
                        _mm(kd[:, kt * 128 : (kt + 1) * 128]),
                        _mm(qd[:, q0 + c_mm : q0 + 512]),
                        start=True,
                        stop=True,
                    )
                    pt = pt_pool.tile([128, 512], MM_DT, tag="pt")
                    if j >= 0:
                        # triangle mask for the diagonal 128-col window
                        nc.vector.tensor_tensor(
                            st[:, c_exp : c_exp + 128],
                            st[:, c_exp : c_exp + 128],
                            mask_sb[:],
                            mybir.AluOpType.add,
                        )
                    nc.scalar.activation(
                        pt[:, c_exp:512],
                        st[:, c_exp:512],
                        mybir.ActivationFunctionType.Exp,
                        scale=0.125,
                    )
                    return pt, c_exp

                def emit_av(kt, pt, c_av):
                    nc.tensor.matmul(
                        ot[:, c_av:512],
                        _mm(vo_sb[kt][:, h * 128 : (h + 1) * 128]),
                        _mm(pt[:, c_av:512]),
                        start=(kt == 0),
                        stop=(kt == n_kt - 1),
                    )

                pending = []
                for kt in range(n_kt):
                    pt, c_av = emit_st_exp(kt)
                    pending.append((kt, pt, c_av))
                    if len(pending) > 3:
                        emit_av(*pending.pop(0))
                for p in pending:
                    emit_av(*p)

                # normalize + V bias, write y^T slice.
                # The denominator row is bounced through DRAM to fold it to
                # [128, 4] so the (slow, ~8 cyc/elem) exact reciprocal runs on
                # all 128 lanes, then unfolded + partition-broadcast back.
                sums = rb_pool.tile([1, 512], F32, tag="sums")
                nc.scalar.copy(sums[:], ot[64:65, :])
                rc_d = dram.tile([1, 512], F32, tag="rc_d")
                nc.sync.dma_start(rc_d[:], sums[:])
                r4 = rb_pool.tile([128, 4], F32, tag="r4")
                nc.sync.dma_start(r4[:], rc_d[0, :].rearrange("(p o) -> p o", p=128))
                nc.vector.reciprocal(r4[:], r4[:])
                rc2_d = dram.tile([1, 512], F32, tag="rc2_d")
                nc.sync.dma_start(
                    rc2_d[0, :].rearrange("(p o) -> p o", p=128), r4[:]
                )
                rb = rb_pool.tile([64, 512], F32, tag="rb")
                nc.sync.dma_start(rb[:], rc2_d[:].to_broadcast((64, 512)))
                yslice = yt_sb[i][jb : jb + 64, q0 : q0 + 512]
                nc.vector.tensor_tensor(
                    yslice,
                    ot[0:64, :],
                    rb[:],
                    mybir.AluOpType.mult,
                )
                nc.vector.tensor_scalar(
                    yslice,
                    yslice,
                    bv_sb[jb : jb + 64, h // 2 : h // 2 + 1],
                    None,
                    mybir.AluOpType.add,
                )

            # partial c_proj for this q-block's t-tiles: dense full-array
            # matmuls interleaved into the exp-bound attention stretch.
            for tt in range(qb * 4, qb * 4 + 4):
                for nb in range(C // 512):
                    ps = p_pr.tile([128, 512], F32, tag="pq")
                    for g in range(HD // 128):
                        nc.tensor.matmul(
                            ps[:],
                            _mm(yt_sb[g][:, tt * 128 : (tt + 1) * 128]),
                            _mm(wp_sb[:, g, nb * 512 : (nb + 1) * 512]),
                            start=(g == 0),
                            stop=(g == HD // 128 - 1),
                        )
                    ob = ob_pool.tile([128, 512], F32, tag="ob")
                    nc.vector.tensor_copy(ob[:], ps[:])
                    nc.sync.dma_start(
                        out[tt * 128 : (tt + 1) * 128, nb * 512 : (nb + 1) * 512],
                        ob[:],
                    )


def _get_nc():
    key = str(MM_DT)
    if key not in _NC_CACHE:
        _NC_CACHE[key] = _build_nc()
    return _NC_CACHE[key]


def _dup_bias(b):
    # [NH*64] -> [128, NH]: head h's 64 biases replicated on both halves
    m = b.reshape(NH, 64).T  # [64, NH]
    return np.ascontiguousarray(np.vstack([m, m]).astype(np.float32))


def kernel(x, Wqkv, bqkv, Wproj, bproj):
    global LAST_RESULT
    x = np.asarray(x, dtype=np.float32)
    Wqkv = np.asarray(Wqkv, dtype=np.float32)
    bqkv = np.asarray(bqkv, dtype=np.float32)
    Wproj = np.asarray(Wproj, dtype=np.float32)
    bproj = np.asarray(bproj, dtype=np.float32)

    nc = _get_nc()
    in_maps = []
    for core in range(N_CORES):
        b, hg = core // HG, core % HG
        cs, ce = hg * HD, (hg + 1) * HD
        in_maps.append(
            {
                "xT": np.ascontiguousarray(x[b].T.astype(MM_NP)),
                "wq": np.ascontiguousarray(Wqkv[:, cs:ce].astype(MM_NP)),
                "wk": np.ascontiguousarray(Wqkv[:, C + cs : C + ce].astype(MM_NP)),
                "wv": np.ascontiguousarray(
                    Wqkv[:, 2 * C + cs : 2 * C + ce].astype(MM_NP)
                ),
                "bq": _dup_bias(bqkv[cs:ce]),
                "bk": np.ascontiguousarray(
                    bqkv[C + cs : C + ce].reshape(2, 128).T.astype(np.float32)
                ),
                "bv": np.ascontiguousarray(bqkv[2 * C + cs : 2 * C + ce]),
                "wp": np.ascontiguousarray(Wproj[cs:ce, :].astype(MM_NP)),
            }
        )

    res = run_bass_kernel_spmd(
        nc, in_maps, core_ids=list(range(N_CORES)), trace=TRACE
    )
    LAST_RESULT = res

    outp = np.empty((B, T, C), dtype=np.float32)
    for b in range(B):
        acc = res.results[b * HG]["out"].astype(np.float32).copy()
        for hg in range(1, HG):
            acc += res.results[b * HG + hg]["out"]
        outp[b] = acc + bproj
    return outp
